# revision 34
# baseline (speedup 1.0000x reference)
"""MoE block (small MLP on all-token-complement, big widened MLP on masked tokens)
as an 8-core Trainium2 Bass/Tile kernel.

Strategy: host-side routing + data parallelism.  The reference computes BOTH
experts densely on every token and selects with the mask; mathematically only
the selected expert's output is needed per token.  We gather big-expert tokens
(mask=True) and small-expert tokens (mask=False) on the host, deal them evenly
across the 8 NeuronCores, run both experts' MLPs on their respective token
shards (dense matmuls in bf16, fp32 accumulation), and scatter back.

Per-core layouts keep the contraction dim on SBUF partitions:
  x   : [d, t]   (d-chunks of 128 on partitions, tokens on the free dim)
  h   : [f, t]   (fc output produced directly in proj's required layout)
  out : [d, t]   (transposed back on the host)
so no on-chip transposes are needed anywhere.

Perf model (measured): each 128x128-stationary matmul slice costs
stream_cols/2.4GHz + ~10ns fixed (LDWEIGHTS hides under streams >= ~110
cols).  Token blocks are therefore capped at 512/core (one PSUM bank,
one slice per weight chunk); the few tokens above 8*512 per expert are
computed on the host in fp32 (identical math, ~0.4% of tokens) instead
of forcing every big matmul into two half-width slices.
"""

import math

import numpy as np
import ml_dtypes

try:
    from scipy.special import erf as _erf
except ImportError:          # exact-math fallback, only used for <100 tokens
    import math

    def _erf(a):
        return np.vectorize(math.erf, otypes=[np.float32])(a)

import concourse.bass as bass
import concourse.mybir as mybir
import concourse.tile as tile
from concourse import bacc
from concourse.bass_utils import run_bass_kernel_spmd

BF16 = ml_dtypes.bfloat16
N_CORES = 8
D_MODEL = 1024
D_FF_S = 4096
D_FF_B = 16384
KD = D_MODEL // 128        # 8 contraction chunks for fc
G = 8                      # f-chunks (of 128) per weight group
MAX_BLK = 512              # PSUM bank limit (fp32 free dim)

_nc_cache = {}
_weights_cache = {}


def _make_runner(nc):
    """Cached equivalent of bass2jax.run_bass_via_pjrt's 8-core path: build
    the jitted shard_map once per compiled Bass program so repeat kernel()
    calls reuse the traced executable instead of recompiling."""
    import jax
    from jax.experimental.shard_map import shard_map
    from jax.sharding import Mesh, PartitionSpec
    from concourse import bass2jax, mybir as _mybir

    bass2jax.install_neuronx_cc_hook()
    assert nc.dbg_addr is None
    partition_name = (nc.partition_id_tensor.name
                      if nc.partition_id_tensor else None)

    in_names, out_names, out_avals, zero_outs = [], [], [], []
    for alloc in nc.m.functions[0].allocations:
        if not isinstance(alloc, _mybir.MemoryLocationSet):
            continue
        name = alloc.memorylocations[0].name
        if alloc.kind == "ExternalInput":
            if name != partition_name:
                in_names.append(name)
        elif alloc.kind == "ExternalOutput":
            shape = tuple(alloc.tensor_shape)
            dtype = _mybir.dt.np(alloc.dtype)
            out_names.append(name)
            out_avals.append(jax.core.ShapedArray(shape, dtype))
            zero_outs.append(np.zeros(shape, dtype))
    n_params = len(in_names)
    all_names = in_names + out_names
    if partition_name is not None:
        all_names = all_names + [partition_name]
    donate = tuple(range(n_params, n_params + len(out_names)))

    def _body(*args):
        operands = list(args)
        if partition_name is not None:
            operands.append(bass2jax.partition_id_tensor())
        return tuple(bass2jax._bass_exec_p.bind(
            *operands,
            out_avals=tuple(out_avals),
            in_names=tuple(all_names),
            out_names=tuple(out_names),
            lowering_input_output_aliases=(),
            sim_require_finite=True,
            sim_require_nnan=True,
            nc=nc,
        ))

    devices = jax.devices()[:N_CORES]
    mesh = Mesh(np.asarray(devices), ("core",))
    nio = n_params + len(out_names)
    sharded = jax.jit(
        shard_map(_body, mesh=mesh,
                  in_specs=(PartitionSpec("core"),) * nio,
                  out_specs=(PartitionSpec("core"),) * len(out_names),
                  check_rep=False),
        donate_argnums=donate, keep_unused=True)

    sharding = jax.sharding.NamedSharding(mesh, PartitionSpec("core"))
    static_cache = {}

    def run(in_maps, static_key=None):
        # per-core-identical weight tensors are device_put once and reused
        concat_in = []
        for name in in_names:
            vals = [np.asarray(in_maps[c][name]) for c in range(N_CORES)]
            static = static_key is not None and all(
                v is vals[0] for v in vals[1:])
            ck = (static_key, name)
            if static and ck in static_cache:
                concat_in.append(static_cache[ck])
                continue
            arr = np.concatenate(vals, axis=0)
            if static:
                arr = jax.device_put(arr, sharding)
                if len(static_cache) > 40:
                    static_cache.clear()
                static_cache[ck] = arr
            concat_in.append(arr)
        concat_zeros = [
            np.zeros((N_CORES * z.shape[0], *z.shape[1:]), z.dtype)
            for z in zero_outs
        ]
        # land all transfers before launching so no core executes while
        # other cores' input DMAs still contend for HBM
        concat_in = [a if isinstance(a, jax.Array) else
                     jax.device_put(a, sharding) for a in concat_in]
        concat_zeros = [jax.device_put(z, sharding) for z in concat_zeros]
        for a in concat_in + concat_zeros:
            a.block_until_ready()
        out_arrs = sharded(*concat_in, *concat_zeros)
        return [
            {name: np.asarray(out_arrs[i]).reshape(
                N_CORES, *out_avals[i].shape)[c]
             for i, name in enumerate(out_names)}
            for c in range(N_CORES)
        ]

    return run


def _split(n):
    """Device takes at most 512 tokens/core (one PSUM-bank-wide block);
    anything beyond 8*512 for an expert is computed on the host."""
    dev_n = min(n, N_CORES * MAX_BLK)
    cap = max(1, math.ceil(dev_n / N_CORES))
    return dev_n, cap


def _build_nc(tb, ts):
    fcb = D_FF_B // 128
    fcs = D_FF_S // 128
    ngb = fcb // G
    ngs = fcs // G

    nc = bacc.Bacc("TRN2", target_bir_lowering=False, debug=False,
                   num_devices=N_CORES, enable_partition_id=False)
    dt = mybir.dt

    # x is packed as four quarter-tensors of 2 d-chunks each; each quarter
    # is one DMA (contiguous 2KB/partition rows) and the four quarters ride
    # four different DMA queues so the first fc chain's operands land in
    # parallel instead of serializing behind one ~95GB/s startup queue.
    xb = nc.dram_tensor("xb", [4, 128, 2, tb], dt.bfloat16, kind="ExternalInput").ap()
    xs = nc.dram_tensor("xs", [4, 128, 2, ts], dt.bfloat16, kind="ExternalInput").ap()
    # first fc weight chunk (big expert, fl=0) duplicated in k-split layout
    # so the very first matmul is gated on a 128KB transfer, not 2MB.
    wfc00 = nc.dram_tensor("wfc00", [2, 128, 4, 128], dt.bfloat16, kind="ExternalInput").ap()
    wfcb = nc.dram_tensor("wfcb", [ngb, 128, G, KD, 128], dt.bfloat16, kind="ExternalInput").ap()
    wpjb = nc.dram_tensor("wpjb", [ngb, 128, 8, G, 128], dt.bfloat16, kind="ExternalInput").ap()
    wfcs = nc.dram_tensor("wfcs", [ngs, 128, G, KD, 128], dt.bfloat16, kind="ExternalInput").ap()
    wpjs = nc.dram_tensor("wpjs", [ngs, 128, 8, G, 128], dt.bfloat16, kind="ExternalInput").ap()
    bfcb = nc.dram_tensor("bfcb", [128, fcb], dt.float32, kind="ExternalInput").ap()
    bfcs = nc.dram_tensor("bfcs", [128, fcs], dt.float32, kind="ExternalInput").ap()
    bpj = nc.dram_tensor("bpj", [128, 2, 8], dt.float32, kind="ExternalInput").ap()
    ob = nc.dram_tensor("ob", [D_MODEL, tb], dt.float32, kind="ExternalOutput").ap()
    os_ = nc.dram_tensor("os", [D_MODEL, ts], dt.float32, kind="ExternalOutput").ap()

    gelu = mybir.ActivationFunctionType.Gelu
    ident = mybir.ActivationFunctionType.Identity

    with tile.TileContext(nc) as tc:
        with (
            tc.tile_pool(name="xpool", bufs=4) as xpool,
            tc.tile_pool(name="wfc00p", bufs=2) as wfc00_pool,
            tc.tile_pool(name="wfc0", bufs=G) as wfc0_pool,
            tc.tile_pool(name="wfc", bufs=2) as wfc_pool,
            tc.tile_pool(name="wpj", bufs=2) as wpj_pool,
            tc.tile_pool(name="hpool", bufs=4 * G) as h_pool,
            tc.tile_pool(name="opool", bufs=16) as out_pool,
            tc.tile_pool(name="bias", bufs=1) as bias_pool,
            tc.tile_pool(name="ph", bufs=5, space="PSUM") as psum_h,
            tc.tile_pool(name="po", bufs=3, space="PSUM") as psum_o,
        ):
            # DMA-issue engine split (issues cost ~0.7-0.9us each and
            # serialize per engine): weights on the Sync HWDGE ring, x on
            # the GpSimd SWDGE path, biases on the Scalar ring.  Final
            # output DMAs alternate Scalar/GpSimd so the drain overlaps
            # the last proj group's compute.

            # Startup is aggregate-HBM-bound: only bytes the first fc group
            # needs may transfer in the first ~12us.  The first chain's gate
            # (wfc00[0] + x k0/k1) rides Sync (the earliest-starting queue);
            # the other x quarters go on GpSimd/Scalar; xs is loaded only
            # after the big expert's instructions are emitted.
            # PE P-state warmup: the tensor engine ramps 0.65 -> 1.2 ->
            # 2.4GHz over ~3us of continuous execution; without this the
            # first ~14 real matmuls run at half clock.  Matmul a memset
            # scratch tile while the x/weight DMAs are still in flight.
            warm = xpool.tile([128, 128], dt.bfloat16, tag="warm",
                              name="warm_sb", bufs=1)
            nc.gpsimd.memset(warm[:], 0.0)
            # enough back-to-back warmup to bridge until x lands (~13.5us):
            # an idle gap between warmup and the first real matmul would
            # drop the P-state right back down.
            pwarm = psum_h.tile([128, tb], dt.float32, tag="ph")
            for _ in range(56):
                nc.tensor.matmul(pwarm[:, 0:128], warm[:], warm[:],
                                 start=True, stop=True)

            def load_x(x_ap, tcap, prefix, engs):
                quarters = []
                for q in range(4):
                    xt = xpool.tile([128, 2, tcap], dt.bfloat16,
                                    tag=f"x{prefix}", name=f"x_{prefix}{q}",
                                    bufs=4)
                    engs[q].dma_start(xt[:], x_ap[q])
                    quarters.append(xt)
                return lambda k: quarters[k // 2][:, k % 2, :]

            # x rides the two HWDGE queues only (Sync/Scalar) — the GpSimd
            # SWDGE path is consistently the slowest to deliver at startup
            wfc00_tiles = [wfc00_pool.tile([128, 4, 128], dt.bfloat16,
                                           tag="wfc00", name=f"wfc00_{h}")
                           for h in range(2)]
            nc.sync.dma_start(wfc00_tiles[0][:], wfc00[0])
            xb_sb = load_x(xb, tb, "b",
                           [nc.sync, nc.scalar, nc.sync, nc.scalar])
            nc.sync.dma_start(wfc00_tiles[1][:], wfc00[1])

            # biases after the x quarters (first gelu trails the first x
            # use by ~2us)
            bias_tiles = {}
            for key, ap, w in (("bfcb", bfcb, fcb), ("bfcs", bfcs, fcs)):
                bias_tiles[key] = bias_pool.tile([128, w], dt.float32,
                                                 tag=key, name=f"{key}_sb")
                nc.scalar.dma_start(bias_tiles[key][:], ap)
            bias_tiles["bpj"] = bias_pool.tile([128, 2, 8], dt.float32,
                                               tag="bpj", name="bpj_sb")
            nc.scalar.dma_start(bias_tiles["bpj"][:], bpj)

            # rest of the first fc weight group (fl=1..7), one tile per fl.
            # All on Sync: spreading these across Scalar/GpSimd was tried
            # and regressed ~8us — the SWDGE path delivers far too slowly
            # at startup, and Sync's in-order delivery already matches the
            # chains' consumption order.
            wfc0_tiles = [None] + [
                wfc0_pool.tile([128, KD, 128], dt.bfloat16,
                               tag="wfc0", name=f"wfc0_{fl}")
                for fl in range(1, G)]
            for fl in range(1, G):
                nc.sync.dma_start(wfc0_tiles[fl][:], wfcb[0, :, fl])

            xs_sb = None

            def load_xs():
                # the Tile scheduler hoists dependency-free DMAs to t=0,
                # where these 1MB of small-expert loads would steal startup
                # HBM bandwidth from the first fc group's x/weight gates.
                # A manual wait keeps them out of the startup window; they
                # still land ~300us before the small expert needs them.
                nonlocal xs_sb
                with tc.tile_wait_until(0.12):
                    xs_sb = load_x(xs, ts, "s",
                                   [nc.gpsimd, nc.scalar, nc.gpsimd, nc.scalar])

            def expert(x_sb, w_fc_ap, w_pj_ap, bfc_key, bpj_col, out_ap,
                       tcap, ng, first, out_rings):
                out_sb = [out_pool.tile([128, tcap], dt.float32,
                                        tag=f"out{bpj_col}", bufs=8,
                                        name=f"out_{bpj_col}{d}")
                          for d in range(8)]
                for fg in range(ng):
                    if first and fg == 0:
                        def wfc_sl(fl, k):
                            if fl == 0:
                                return wfc00_tiles[k // 4][:, k % 4, :]
                            return wfc0_tiles[fl][:, k, :]
                    else:
                        wfc_t = wfc_pool.tile([128, G, KD, 128], dt.bfloat16,
                                              tag="wfc")
                        nc.sync.dma_start(wfc_t[:], w_fc_ap[fg])
                        wfc_sl = lambda fl, k, t=wfc_t: t[:, fl, k, :]
                    # wpj stays on Sync: moving it to Scalar was tried and
                    # regressed ~8us (Sync is the fastest delivery queue;
                    # splitting weight groups across queues only slows them)
                    wpj_t = wpj_pool.tile([128, 8, G, 128], dt.bfloat16,
                                          tag="wpj")
                    nc.sync.dma_start(wpj_t[:], w_pj_ap[fg])
                    wpj_sl = lambda dd, fl, t=wpj_t: t[:, dd, fl, :]
                    bfc_sb = bias_tiles[bfc_key]
                    bpj_sb = bias_tiles["bpj"]
                    h = {}
                    for fl in range(G):
                        ph = psum_h.tile([128, tcap], dt.float32, tag="ph")
                        for k in range(KD):
                            nc.tensor.matmul(ph[:], wfc_sl(fl, k), x_sb(k),
                                             start=(k == 0), stop=(k == KD - 1))
                        ht = h_pool.tile([128, tcap], dt.bfloat16, tag="h")
                        fc = fg * G + fl
                        nc.scalar.activation(ht[:], ph[:], gelu,
                                             bias=bfc_sb[:, fc:fc + 1])
                        h[fl] = ht
                    for d in range(8):
                        po = psum_o.tile([128, tcap], dt.float32, tag="po")
                        for fl in range(G):
                            nc.tensor.matmul(po[:], wpj_sl(d, fl), h[fl][:],
                                             start=(fl == 0), stop=(fl == G - 1))
                        if fg == 0:
                            nc.scalar.activation(
                                out_sb[d][:], po[:], ident,
                                bias=bpj_sb[:, bpj_col, d:d + 1])
                        else:
                            nc.vector.tensor_add(out_sb[d][:],
                                                 out_sb[d][:], po[:])
                        if fg == ng - 1:
                            # column-split DMAs on separate rings so the
                            # final drain overlaps the last d's accumulation
                            nr = len(out_rings)
                            lo = 0
                            for j in range(nr):
                                hi = (tcap * (j + 1)) // nr
                                out_rings[(d + j) % nr].dma_start(
                                    out_ap[d * 128:(d + 1) * 128, lo:hi],
                                    out_sb[d][:, lo:hi])
                                lo = hi

            expert(xb_sb, wfcb, wpjb, "bfcb", 0, ob, tb, ngb, True,
                   [nc.scalar, nc.gpsimd])
            load_xs()
            # sync is idle after the last small weight group: use it for
            # the final drain
            expert(xs_sb, wfcs, wpjs, "bfcs", 1, os_, ts, ngs, False,
                   [nc.scalar, nc.gpsimd, nc.sync])

    nc.compile()
    return nc


def _prep_weights(w_fc_s, b_fc_s, w_proj_s, b_proj_s, w_fc_b, b_fc_b,
                  w_proj_b, b_proj_b):
    key = (id(w_fc_s), id(w_fc_b), id(w_proj_s), id(w_proj_b))
    hit = _weights_cache.get(key)
    if hit is not None:
        return hit

    def fc_re(w, f):
        ng = f // 128 // G
        w16 = np.asarray(w, np.float32).astype(BF16)
        r = w16.reshape(ng, G, 128, KD, 128).transpose(0, 4, 1, 3, 2)
        return np.ascontiguousarray(r)

    def pj_re(w, f):
        ng = f // 128 // G
        w16 = np.asarray(w, np.float32).astype(BF16)
        r = w16.reshape(8, 128, ng, G, 128).transpose(2, 4, 0, 3, 1)
        return np.ascontiguousarray(r)

    def b_re(b, f):
        return np.ascontiguousarray(
            np.asarray(b, np.float32).reshape(f // 128, 128).T)

    bpj = np.stack([b_re(b_proj_b, D_MODEL), b_re(b_proj_s, D_MODEL)], axis=1)
    wfcb = fc_re(w_fc_b, D_FF_B)
    # [128(d2), KD, 128(f)] -> [2, 128, 4, 128] k-split copy of (fg=0, fl=0)
    wfc00 = np.ascontiguousarray(
        wfcb[0, :, 0].reshape(128, 2, 4, 128).transpose(1, 0, 2, 3))
    out = {
        "wfcb": wfcb,
        "wfc00": wfc00,
        "wpjb": pj_re(w_proj_b, D_FF_B),
        "wfcs": fc_re(w_fc_s, D_FF_S),
        "wpjs": pj_re(w_proj_s, D_FF_S),
        "bfcb": b_re(b_fc_b, D_FF_B),
        "bfcs": b_re(b_fc_s, D_FF_S),
        "bpj": np.ascontiguousarray(bpj),
    }
    _weights_cache.clear()
    _weights_cache[key] = out
    return out


def _host_mlp(x, w_fc, b_fc, w_proj, b_proj):
    """Exact fp32 MLP for the handful of tokens beyond the device cap."""
    hpre = x @ w_fc.T + b_fc
    hact = 0.5 * hpre * (1.0 + _erf(hpre / np.sqrt(2.0).astype(np.float32)))
    return hact.astype(np.float32) @ w_proj.T + b_proj


def kernel(x, mask, w_fc_s, b_fc_s, w_proj_s, b_proj_s,
           w_fc_b, b_fc_b, w_proj_b, b_proj_b, _profile=None):
    x = np.asarray(x, np.float32)
    mask = np.asarray(mask, bool)
    n_tok = x.shape[0] * x.shape[1]
    xf = x.reshape(n_tok, D_MODEL)
    mf = mask.reshape(n_tok)

    big_idx = np.nonzero(mf)[0]
    small_idx = np.nonzero(~mf)[0]
    dev_nb, tb = _split(len(big_idx))
    dev_ns, ts = _split(len(small_idx))

    def assign(idx, cap):
        a = np.full(N_CORES * cap, -1, np.int64)
        a[:len(idx)] = idx
        return a.reshape(N_CORES, cap)

    a_b = assign(big_idx[:dev_nb], tb)
    a_s = assign(small_idx[:dev_ns], ts)

    xf16 = xf.astype(BF16)

    def tok_arrays(a, cap):
        t = xf16[np.maximum(a, 0)]                       # [cores, cap, D]
        t = (t.reshape(N_CORES, cap, 4, 2, 128)
             .transpose(0, 2, 4, 3, 1))                  # [cores, 4, 128, 2, cap]
        return np.ascontiguousarray(t)

    xb_all = tok_arrays(a_b, tb)
    xs_all = tok_arrays(a_s, ts)

    wd = _prep_weights(w_fc_s, b_fc_s, w_proj_s, b_proj_s,
                       w_fc_b, b_fc_b, w_proj_b, b_proj_b)

    nckey = (tb, ts)
    ent = _nc_cache.get(nckey)
    if ent is None:
        _nc_cache.clear()
        nc = _build_nc(*nckey)
        ent = (nc, _make_runner(nc))
        _nc_cache[nckey] = ent
    nc, runner = ent

    in_maps = [dict(wd, xb=xb_all[c], xs=xs_all[c]) for c in range(N_CORES)]
    if _profile:
        res = run_bass_kernel_spmd(nc, in_maps, core_ids=list(range(N_CORES)),
                                   **dict(_profile))
        results = res.results
        _profile["results"] = res
    else:
        results = runner(in_maps, static_key=id(wd))

    out_t = np.empty((D_MODEL, n_tok), np.float32)

    def scatter(name, a):
        o = np.concatenate([results[c][name] for c in range(N_CORES)], axis=1)
        flat = a.reshape(-1)
        valid = flat >= 0
        out_t[:, flat[valid]] = o[:, valid]

    scatter("ob", a_b)
    scatter("os", a_s)

    # leftover tokens (beyond the 8*512 device cap) in exact fp32 on host
    for idx, w_fc, b_fc, w_proj, b_proj in (
            (big_idx[dev_nb:], w_fc_b, b_fc_b, w_proj_b, b_proj_b),
            (small_idx[dev_ns:], w_fc_s, b_fc_s, w_proj_s, b_proj_s)):
        if len(idx):
            out_t[:, idx] = _host_mlp(
                xf[idx], np.asarray(w_fc, np.float32),
                np.asarray(b_fc, np.float32),
                np.asarray(w_proj, np.float32),
                np.asarray(b_proj, np.float32)).T

    return out_t.T.reshape(x.shape)


# revision 36
# speedup vs baseline: 1.0021x; 1.0021x over previous
"""MoE block (small MLP on all-token-complement, big widened MLP on masked tokens)
as an 8-core Trainium2 Bass/Tile kernel.

Strategy: host-side routing + data parallelism.  The reference computes BOTH
experts densely on every token and selects with the mask; mathematically only
the selected expert's output is needed per token.  We gather big-expert tokens
(mask=True) and small-expert tokens (mask=False) on the host, deal them evenly
across the 8 NeuronCores, run both experts' MLPs on their respective token
shards (dense matmuls in bf16, fp32 accumulation), and scatter back.

Per-core layouts keep the contraction dim on SBUF partitions:
  x   : [d, t]   (d-chunks of 128 on partitions, tokens on the free dim)
  h   : [f, t]   (fc output produced directly in proj's required layout)
  out : [d, t]   (transposed back on the host)
so no on-chip transposes are needed anywhere.

Perf model (measured): each 128x128-stationary matmul slice costs
stream_cols/2.4GHz + ~10ns fixed (LDWEIGHTS hides under streams >= ~110
cols).  Token blocks are therefore capped at 512/core (one PSUM bank,
one slice per weight chunk); the few tokens above 8*512 per expert are
computed on the host in fp32 (identical math, ~0.4% of tokens) instead
of forcing every big matmul into two half-width slices.
"""

import math

import numpy as np
import ml_dtypes

try:
    from scipy.special import erf as _erf
except ImportError:          # exact-math fallback, only used for <100 tokens
    import math

    def _erf(a):
        return np.vectorize(math.erf, otypes=[np.float32])(a)

import concourse.bass as bass
import concourse.mybir as mybir
import concourse.tile as tile
from concourse import bacc
from concourse.bass_utils import run_bass_kernel_spmd

BF16 = ml_dtypes.bfloat16
N_CORES = 8
D_MODEL = 1024
D_FF_S = 4096
D_FF_B = 16384
KD = D_MODEL // 128        # 8 contraction chunks for fc
G = 8                      # f-chunks (of 128) per weight group
MAX_BLK = 512              # PSUM bank limit (fp32 free dim)

_nc_cache = {}
_weights_cache = {}


def _make_runner(nc):
    """Cached equivalent of bass2jax.run_bass_via_pjrt's 8-core path: build
    the jitted shard_map once per compiled Bass program so repeat kernel()
    calls reuse the traced executable instead of recompiling."""
    import jax
    from jax.experimental.shard_map import shard_map
    from jax.sharding import Mesh, PartitionSpec
    from concourse import bass2jax, mybir as _mybir

    bass2jax.install_neuronx_cc_hook()
    assert nc.dbg_addr is None
    partition_name = (nc.partition_id_tensor.name
                      if nc.partition_id_tensor else None)

    in_names, out_names, out_avals, zero_outs = [], [], [], []
    for alloc in nc.m.functions[0].allocations:
        if not isinstance(alloc, _mybir.MemoryLocationSet):
            continue
        name = alloc.memorylocations[0].name
        if alloc.kind == "ExternalInput":
            if name != partition_name:
                in_names.append(name)
        elif alloc.kind == "ExternalOutput":
            shape = tuple(alloc.tensor_shape)
            dtype = _mybir.dt.np(alloc.dtype)
            out_names.append(name)
            out_avals.append(jax.core.ShapedArray(shape, dtype))
            zero_outs.append(np.zeros(shape, dtype))
    n_params = len(in_names)
    all_names = in_names + out_names
    if partition_name is not None:
        all_names = all_names + [partition_name]
    donate = tuple(range(n_params, n_params + len(out_names)))

    def _body(*args):
        operands = list(args)
        if partition_name is not None:
            operands.append(bass2jax.partition_id_tensor())
        return tuple(bass2jax._bass_exec_p.bind(
            *operands,
            out_avals=tuple(out_avals),
            in_names=tuple(all_names),
            out_names=tuple(out_names),
            lowering_input_output_aliases=(),
            sim_require_finite=True,
            sim_require_nnan=True,
            nc=nc,
        ))

    devices = jax.devices()[:N_CORES]
    mesh = Mesh(np.asarray(devices), ("core",))
    nio = n_params + len(out_names)
    sharded = jax.jit(
        shard_map(_body, mesh=mesh,
                  in_specs=(PartitionSpec("core"),) * nio,
                  out_specs=(PartitionSpec("core"),) * len(out_names),
                  check_rep=False),
        donate_argnums=donate, keep_unused=True)

    sharding = jax.sharding.NamedSharding(mesh, PartitionSpec("core"))
    static_cache = {}

    def run(in_maps, static_key=None):
        # per-core-identical weight tensors are device_put once and reused
        concat_in = []
        for name in in_names:
            vals = [np.asarray(in_maps[c][name]) for c in range(N_CORES)]
            static = static_key is not None and all(
                v is vals[0] for v in vals[1:])
            ck = (static_key, name)
            if static and ck in static_cache:
                concat_in.append(static_cache[ck])
                continue
            arr = np.concatenate(vals, axis=0)
            if static:
                arr = jax.device_put(arr, sharding)
                if len(static_cache) > 40:
                    static_cache.clear()
                static_cache[ck] = arr
            concat_in.append(arr)
        concat_zeros = [
            np.zeros((N_CORES * z.shape[0], *z.shape[1:]), z.dtype)
            for z in zero_outs
        ]
        # land all transfers before launching so no core executes while
        # other cores' input DMAs still contend for HBM
        concat_in = [a if isinstance(a, jax.Array) else
                     jax.device_put(a, sharding) for a in concat_in]
        concat_zeros = [jax.device_put(z, sharding) for z in concat_zeros]
        for a in concat_in + concat_zeros:
            a.block_until_ready()
        out_arrs = sharded(*concat_in, *concat_zeros)
        return [
            {name: np.asarray(out_arrs[i]).reshape(
                N_CORES, *out_avals[i].shape)[c]
             for i, name in enumerate(out_names)}
            for c in range(N_CORES)
        ]

    return run


def _split(n):
    """Device takes at most 512 tokens/core (one PSUM-bank-wide block);
    anything beyond 8*512 for an expert is computed on the host."""
    dev_n = min(n, N_CORES * MAX_BLK)
    cap = max(1, math.ceil(dev_n / N_CORES))
    return dev_n, cap


def _build_nc(tb, ts):
    fcb = D_FF_B // 128
    fcs = D_FF_S // 128
    ngb = fcb // G
    ngs = fcs // G

    nc = bacc.Bacc("TRN2", target_bir_lowering=False, debug=False,
                   num_devices=N_CORES, enable_partition_id=False)
    dt = mybir.dt

    # x is packed as four quarter-tensors of 2 d-chunks each; each quarter
    # is one DMA (contiguous 2KB/partition rows) and the four quarters ride
    # four different DMA queues so the first fc chain's operands land in
    # parallel instead of serializing behind one ~95GB/s startup queue.
    xb = nc.dram_tensor("xb", [4, 128, 2, tb], dt.bfloat16, kind="ExternalInput").ap()
    xs = nc.dram_tensor("xs", [4, 128, 2, ts], dt.bfloat16, kind="ExternalInput").ap()
    # first fc weight chunk (big expert, fl=0) duplicated in k-split layout
    # so the very first matmul is gated on a 128KB transfer, not 2MB.
    wfc00 = nc.dram_tensor("wfc00", [2, 128, 4, 128], dt.bfloat16, kind="ExternalInput").ap()
    wfcb = nc.dram_tensor("wfcb", [ngb, 128, G, KD, 128], dt.bfloat16, kind="ExternalInput").ap()
    wpjb = nc.dram_tensor("wpjb", [ngb, 128, 8, G, 128], dt.bfloat16, kind="ExternalInput").ap()
    wfcs = nc.dram_tensor("wfcs", [ngs, 128, G, KD, 128], dt.bfloat16, kind="ExternalInput").ap()
    wpjs = nc.dram_tensor("wpjs", [ngs, 128, 8, G, 128], dt.bfloat16, kind="ExternalInput").ap()
    bfcb = nc.dram_tensor("bfcb", [128, fcb], dt.float32, kind="ExternalInput").ap()
    bfcs = nc.dram_tensor("bfcs", [128, fcs], dt.float32, kind="ExternalInput").ap()
    bpj = nc.dram_tensor("bpj", [128, 2, 8], dt.float32, kind="ExternalInput").ap()
    ob = nc.dram_tensor("ob", [D_MODEL, tb], dt.float32, kind="ExternalOutput").ap()
    os_ = nc.dram_tensor("os", [D_MODEL, ts], dt.float32, kind="ExternalOutput").ap()

    gelu = mybir.ActivationFunctionType.Gelu
    ident = mybir.ActivationFunctionType.Identity

    with tile.TileContext(nc) as tc:
        with (
            tc.tile_pool(name="xpool", bufs=4) as xpool,
            tc.tile_pool(name="wfc00p", bufs=2) as wfc00_pool,
            tc.tile_pool(name="wfc0", bufs=G) as wfc0_pool,
            tc.tile_pool(name="wfc", bufs=2) as wfc_pool,
            tc.tile_pool(name="wpj", bufs=2) as wpj_pool,
            tc.tile_pool(name="hpool", bufs=4 * G) as h_pool,
            tc.tile_pool(name="opool", bufs=16) as out_pool,
            tc.tile_pool(name="bias", bufs=1) as bias_pool,
            tc.tile_pool(name="ph", bufs=4, space="PSUM") as psum_h,
            tc.tile_pool(name="po", bufs=4, space="PSUM") as psum_o,
        ):
            # DMA-issue engine split (issues cost ~0.7-0.9us each and
            # serialize per engine): weights on the Sync HWDGE ring, x on
            # the GpSimd SWDGE path, biases on the Scalar ring.  Final
            # output DMAs alternate Scalar/GpSimd so the drain overlaps
            # the last proj group's compute.

            # Startup is aggregate-HBM-bound: only bytes the first fc group
            # needs may transfer in the first ~12us.  The first chain's gate
            # (wfc00[0] + x k0/k1) rides Sync (the earliest-starting queue);
            # the other x quarters go on GpSimd/Scalar; xs is loaded only
            # after the big expert's instructions are emitted.
            # PE P-state warmup: the tensor engine ramps 0.65 -> 1.2 ->
            # 2.4GHz over ~3us of continuous execution; without this the
            # first ~14 real matmuls run at half clock.  Matmul a memset
            # scratch tile while the x/weight DMAs are still in flight.
            warm = xpool.tile([128, 128], dt.bfloat16, tag="warm",
                              name="warm_sb", bufs=1)
            nc.gpsimd.memset(warm[:], 0.0)
            # enough back-to-back warmup to bridge until x lands (~13.5us):
            # an idle gap between warmup and the first real matmul would
            # drop the P-state right back down.
            pwarm = psum_h.tile([128, tb], dt.float32, tag="ph")
            for _ in range(56):
                nc.tensor.matmul(pwarm[:, 0:128], warm[:], warm[:],
                                 start=True, stop=True)

            def load_x(x_ap, tcap, prefix, engs):
                quarters = []
                for q in range(4):
                    xt = xpool.tile([128, 2, tcap], dt.bfloat16,
                                    tag=f"x{prefix}", name=f"x_{prefix}{q}",
                                    bufs=4)
                    engs[q].dma_start(xt[:], x_ap[q])
                    quarters.append(xt)
                return lambda k: quarters[k // 2][:, k % 2, :]

            # x rides the two HWDGE queues only (Sync/Scalar) — the GpSimd
            # SWDGE path is consistently the slowest to deliver at startup
            wfc00_tiles = [wfc00_pool.tile([128, 4, 128], dt.bfloat16,
                                           tag="wfc00", name=f"wfc00_{h}")
                           for h in range(2)]
            nc.sync.dma_start(wfc00_tiles[0][:], wfc00[0])
            xb_sb = load_x(xb, tb, "b",
                           [nc.sync, nc.scalar, nc.sync, nc.scalar])
            nc.sync.dma_start(wfc00_tiles[1][:], wfc00[1])

            # biases ride the otherwise-idle GpSimd queue: the scheduler
            # hoists dependency-free DMAs to the queue front, so on Scalar
            # they would delay the x quarters that gate the first chains.
            # They are tiny and not needed until the first gelu (~16us).
            bias_tiles = {}
            for key, ap, w in (("bfcb", bfcb, fcb), ("bfcs", bfcs, fcs)):
                bias_tiles[key] = bias_pool.tile([128, w], dt.float32,
                                                 tag=key, name=f"{key}_sb")
                nc.gpsimd.dma_start(bias_tiles[key][:], ap)
            bias_tiles["bpj"] = bias_pool.tile([128, 2, 8], dt.float32,
                                               tag="bpj", name="bpj_sb")
            nc.gpsimd.dma_start(bias_tiles["bpj"][:], bpj)

            # rest of the first fc weight group (fl=1..7), one tile per fl.
            # All on Sync: spreading these across Scalar/GpSimd was tried
            # and regressed ~8us — the SWDGE path delivers far too slowly
            # at startup, and Sync's in-order delivery already matches the
            # chains' consumption order.
            wfc0_tiles = [None] + [
                wfc0_pool.tile([128, KD, 128], dt.bfloat16,
                               tag="wfc0", name=f"wfc0_{fl}")
                for fl in range(1, G)]
            for fl in range(1, G):
                nc.sync.dma_start(wfc0_tiles[fl][:], wfcb[0, :, fl])

            xs_sb = None

            def load_xs():
                # the Tile scheduler hoists dependency-free DMAs to t=0,
                # where these 1MB of small-expert loads would steal startup
                # HBM bandwidth from the first fc group's x/weight gates.
                # A manual wait keeps them out of the startup window; they
                # still land ~300us before the small expert needs them.
                nonlocal xs_sb
                with tc.tile_wait_until(0.12):
                    xs_sb = load_x(xs, ts, "s",
                                   [nc.gpsimd, nc.scalar, nc.gpsimd, nc.scalar])

            def expert(x_sb, w_fc_ap, w_pj_ap, bfc_key, bpj_col, out_ap,
                       tcap, ng, first, out_rings):
                out_sb = [out_pool.tile([128, tcap], dt.float32,
                                        tag=f"out{bpj_col}", bufs=8,
                                        name=f"out_{bpj_col}{d}")
                          for d in range(8)]
                for fg in range(ng):
                    if first and fg == 0:
                        def wfc_sl(fl, k):
                            if fl == 0:
                                return wfc00_tiles[k // 4][:, k % 4, :]
                            return wfc0_tiles[fl][:, k, :]
                    else:
                        wfc_t = wfc_pool.tile([128, G, KD, 128], dt.bfloat16,
                                              tag="wfc")
                        nc.sync.dma_start(wfc_t[:], w_fc_ap[fg])
                        wfc_sl = lambda fl, k, t=wfc_t: t[:, fl, k, :]
                    # wpj stays on Sync: moving it to Scalar was tried and
                    # regressed ~8us (Sync is the fastest delivery queue;
                    # splitting weight groups across queues only slows them)
                    wpj_t = wpj_pool.tile([128, 8, G, 128], dt.bfloat16,
                                          tag="wpj")
                    nc.sync.dma_start(wpj_t[:], w_pj_ap[fg])
                    wpj_sl = lambda dd, fl, t=wpj_t: t[:, dd, fl, :]
                    bfc_sb = bias_tiles[bfc_key]
                    bpj_sb = bias_tiles["bpj"]
                    h = {}
                    for fl in range(G):
                        ph = psum_h.tile([128, tcap], dt.float32, tag="ph")
                        for k in range(KD):
                            nc.tensor.matmul(ph[:], wfc_sl(fl, k), x_sb(k),
                                             start=(k == 0), stop=(k == KD - 1))
                        ht = h_pool.tile([128, tcap], dt.bfloat16, tag="h")
                        fc = fg * G + fl
                        nc.scalar.activation(ht[:], ph[:], gelu,
                                             bias=bfc_sb[:, fc:fc + 1])
                        h[fl] = ht
                    for d in range(8):
                        po = psum_o.tile([128, tcap], dt.float32, tag="po")
                        for fl in range(G):
                            nc.tensor.matmul(po[:], wpj_sl(d, fl), h[fl][:],
                                             start=(fl == 0), stop=(fl == G - 1))
                        if fg == 0:
                            nc.scalar.activation(
                                out_sb[d][:], po[:], ident,
                                bias=bpj_sb[:, bpj_col, d:d + 1])
                        else:
                            nc.vector.tensor_add(out_sb[d][:],
                                                 out_sb[d][:], po[:])
                        if fg == ng - 1:
                            # column-split DMAs on separate rings so the
                            # final drain overlaps the last d's accumulation
                            nr = len(out_rings)
                            lo = 0
                            for j in range(nr):
                                hi = (tcap * (j + 1)) // nr
                                out_rings[(d + j) % nr].dma_start(
                                    out_ap[d * 128:(d + 1) * 128, lo:hi],
                                    out_sb[d][:, lo:hi])
                                lo = hi

            expert(xb_sb, wfcb, wpjb, "bfcb", 0, ob, tb, ngb, True,
                   [nc.scalar, nc.gpsimd])
            load_xs()
            # sync is idle after the last small weight group: use it for
            # the final drain
            expert(xs_sb, wfcs, wpjs, "bfcs", 1, os_, ts, ngs, False,
                   [nc.scalar, nc.gpsimd, nc.sync])

    nc.compile()
    return nc


def _prep_weights(w_fc_s, b_fc_s, w_proj_s, b_proj_s, w_fc_b, b_fc_b,
                  w_proj_b, b_proj_b):
    key = (id(w_fc_s), id(w_fc_b), id(w_proj_s), id(w_proj_b))
    hit = _weights_cache.get(key)
    if hit is not None:
        return hit

    def fc_re(w, f):
        ng = f // 128 // G
        w16 = np.asarray(w, np.float32).astype(BF16)
        r = w16.reshape(ng, G, 128, KD, 128).transpose(0, 4, 1, 3, 2)
        return np.ascontiguousarray(r)

    def pj_re(w, f):
        ng = f // 128 // G
        w16 = np.asarray(w, np.float32).astype(BF16)
        r = w16.reshape(8, 128, ng, G, 128).transpose(2, 4, 0, 3, 1)
        return np.ascontiguousarray(r)

    def b_re(b, f):
        return np.ascontiguousarray(
            np.asarray(b, np.float32).reshape(f // 128, 128).T)

    bpj = np.stack([b_re(b_proj_b, D_MODEL), b_re(b_proj_s, D_MODEL)], axis=1)
    wfcb = fc_re(w_fc_b, D_FF_B)
    # [128(d2), KD, 128(f)] -> [2, 128, 4, 128] k-split copy of (fg=0, fl=0)
    wfc00 = np.ascontiguousarray(
        wfcb[0, :, 0].reshape(128, 2, 4, 128).transpose(1, 0, 2, 3))
    out = {
        "wfcb": wfcb,
        "wfc00": wfc00,
        "wpjb": pj_re(w_proj_b, D_FF_B),
        "wfcs": fc_re(w_fc_s, D_FF_S),
        "wpjs": pj_re(w_proj_s, D_FF_S),
        "bfcb": b_re(b_fc_b, D_FF_B),
        "bfcs": b_re(b_fc_s, D_FF_S),
        "bpj": np.ascontiguousarray(bpj),
    }
    _weights_cache.clear()
    _weights_cache[key] = out
    return out


def _host_mlp(x, w_fc, b_fc, w_proj, b_proj):
    """Exact fp32 MLP for the handful of tokens beyond the device cap."""
    hpre = x @ w_fc.T + b_fc
    hact = 0.5 * hpre * (1.0 + _erf(hpre / np.sqrt(2.0).astype(np.float32)))
    return hact.astype(np.float32) @ w_proj.T + b_proj


def kernel(x, mask, w_fc_s, b_fc_s, w_proj_s, b_proj_s,
           w_fc_b, b_fc_b, w_proj_b, b_proj_b, _profile=None):
    x = np.asarray(x, np.float32)
    mask = np.asarray(mask, bool)
    n_tok = x.shape[0] * x.shape[1]
    xf = x.reshape(n_tok, D_MODEL)
    mf = mask.reshape(n_tok)

    big_idx = np.nonzero(mf)[0]
    small_idx = np.nonzero(~mf)[0]
    dev_nb, tb = _split(len(big_idx))
    dev_ns, ts = _split(len(small_idx))

    def assign(idx, cap):
        a = np.full(N_CORES * cap, -1, np.int64)
        a[:len(idx)] = idx
        return a.reshape(N_CORES, cap)

    a_b = assign(big_idx[:dev_nb], tb)
    a_s = assign(small_idx[:dev_ns], ts)

    xf16 = xf.astype(BF16)

    def tok_arrays(a, cap):
        t = xf16[np.maximum(a, 0)]                       # [cores, cap, D]
        t = (t.reshape(N_CORES, cap, 4, 2, 128)
             .transpose(0, 2, 4, 3, 1))                  # [cores, 4, 128, 2, cap]
        return np.ascontiguousarray(t)

    xb_all = tok_arrays(a_b, tb)
    xs_all = tok_arrays(a_s, ts)

    wd = _prep_weights(w_fc_s, b_fc_s, w_proj_s, b_proj_s,
                       w_fc_b, b_fc_b, w_proj_b, b_proj_b)

    nckey = (tb, ts)
    ent = _nc_cache.get(nckey)
    if ent is None:
        _nc_cache.clear()
        nc = _build_nc(*nckey)
        ent = (nc, _make_runner(nc))
        _nc_cache[nckey] = ent
    nc, runner = ent

    in_maps = [dict(wd, xb=xb_all[c], xs=xs_all[c]) for c in range(N_CORES)]
    if _profile:
        res = run_bass_kernel_spmd(nc, in_maps, core_ids=list(range(N_CORES)),
                                   **dict(_profile))
        results = res.results
        _profile["results"] = res
    else:
        results = runner(in_maps, static_key=id(wd))

    out_t = np.empty((D_MODEL, n_tok), np.float32)

    def scatter(name, a):
        o = np.concatenate([results[c][name] for c in range(N_CORES)], axis=1)
        flat = a.reshape(-1)
        valid = flat >= 0
        out_t[:, flat[valid]] = o[:, valid]

    scatter("ob", a_b)
    scatter("os", a_s)

    # leftover tokens (beyond the 8*512 device cap) in exact fp32 on host
    for idx, w_fc, b_fc, w_proj, b_proj in (
            (big_idx[dev_nb:], w_fc_b, b_fc_b, w_proj_b, b_proj_b),
            (small_idx[dev_ns:], w_fc_s, b_fc_s, w_proj_s, b_proj_s)):
        if len(idx):
            out_t[:, idx] = _host_mlp(
                xf[idx], np.asarray(w_fc, np.float32),
                np.asarray(b_fc, np.float32),
                np.asarray(w_proj, np.float32),
                np.asarray(b_proj, np.float32)).T

    return out_t.T.reshape(x.shape)
